# revision 3
# baseline (speedup 1.0000x reference)
"""TopK sparse autoencoder kernel for Trainium2 (8 NeuronCores, data-parallel).

Problem: B=4096, N_IN=4096, N_LAT=32768, K=64
  pre   = (x - pre_bias) @ encoder + latent_bias          [B, N_LAT]
  latents = top-64-masked relu(pre); full_latents = relu(pre)
  reconstructed(_full) = latents(full) @ decoder + pre_bias

Sharding: data-parallel over batch. Each of 8 cores handles 512 rows with
replicated weights; no collectives.

Per-core device program:
  encode: fp32 GEMM (top-k selection is precision-critical; fp32r/bf16 would
          swap near-threshold selections vs the fp32 reference), bias folded
          in via an appended ones-row (host-side).
  top-k : per 512-wide output chunk, DVE max8 gives top-8 candidates
          (64 chunks x 8 = 512 candidates/row always contain the true
          top-64 up to ~1e-5/row probability); 8 rounds of max8 +
          match_replace on candidates give the 64th-largest threshold t;
          latents = flat * (flat >= t) where flat = relu(pre).
  decode: bf16 GEMMs with latents/full_latents transposed via PE.
"""
import numpy as np
import ml_dtypes

N_CORES = 8
B, N_IN, N_LAT, TOPK = 4096, 4096, 32768, 64

_nc_cache = {}


def build_nc(rows=B // N_CORES, n_in=N_IN, n_lat=N_LAT, k=TOPK):
    import concourse.mybir as mybir
    from concourse import bacc
    from concourse.tile import TileContext
    from concourse.masks import make_identity

    f32, bf16 = mybir.dt.float32, mybir.dt.bfloat16
    Relu = mybir.ActivationFunctionType.Relu
    Copy = mybir.ActivationFunctionType.Copy
    is_ge, mult = mybir.AluOpType.is_ge, mybir.AluOpType.mult

    KC = n_in // 128 + 1          # contraction chunks incl. bias row (padded)
    n_in_pad = KC * 128
    R = rows // 128               # rows-tiles
    C = n_lat // 512              # encode N-chunks
    L = n_lat // 128              # decode contraction chunks
    SW = 1024                     # select slice width
    SC = n_lat // SW
    TPS = SW // 128               # transposes per select slice
    GK = 9                        # encoder K-chunks per stream group
    EG = (KC + GK - 1) // GK      # stream groups per encode N-chunk
    LG = 2                        # decode l-chunks per dec DMA
    NINC = n_in // 512            # nin output chunks
    # decode psum quads: groups of nin chunks processed per pass (2 types each)
    quads = []
    q0 = 0
    while q0 < NINC:
        qn = min(3, NINC - q0)
        quads.append((q0, qn))
        q0 += qn
    rounds = (k + 7) // 8
    assert rows % 128 == 0 and n_lat % 512 == 0 and n_in % 512 == 0
    assert k <= C * 8, "candidate pool must cover k"

    nc = bacc.Bacc()
    xT = nc.dram_tensor("xT", [n_in_pad, rows], f32, kind="ExternalInput")
    encp = nc.dram_tensor("encp", [n_in_pad, n_lat], f32, kind="ExternalInput")
    dec = nc.dram_tensor("dec", [n_lat, n_in], bf16, kind="ExternalInput")
    pb = nc.dram_tensor("pb", [1, n_in], bf16, kind="ExternalInput")
    recon = nc.dram_tensor("recon", [rows, n_in], f32, kind="ExternalOutput")
    lat = nc.dram_tensor("lat", [rows, n_lat], f32, kind="ExternalOutput")
    recon_f = nc.dram_tensor("recon_f", [rows, n_in], f32, kind="ExternalOutput")
    flat = nc.dram_tensor("flat", [rows, n_lat], f32, kind="ExternalOutput")

    with TileContext(nc) as tc:
        with (
            tc.tile_pool(name="glob", bufs=1) as glob,
            tc.tile_pool(name="dram", bufs=1, space="DRAM") as dramp,
        ):
            ident = glob.tile([128, 128], f32, tag="ident")
            make_identity(nc, ident)
            cands = [glob.tile([128, C * 8], f32, tag=f"cand{r}", name=f"cand{r}") for r in range(R)]
            thrs = [glob.tile([128, 1], f32, tag=f"thr{r}", name=f"thr{r}") for r in range(R)]
            scr = dramp.tile([rows, n_lat], f32, tag="scr")  # relu(pre) scratch

            # ---------------- encode ----------------
            with (
                tc.tile_pool(name="xt", bufs=1) as xtp,
                tc.tile_pool(name="encs", bufs=5) as encs,
                tc.tile_pool(name="ev", bufs=4) as evp,
                tc.tile_pool(name="eps", bufs=2, space="PSUM") as eps,
            ):
                xt_sb = xtp.tile([128, KC * rows], f32, tag="xt")
                nc.sync.dma_start(
                    out=xt_sb[:, :].rearrange("p (a m) -> p a m", a=KC),
                    in_=xT[:, :].rearrange("(a p) m -> p a m", p=128),
                )
                for c in range(C):
                    cs = slice(c * 512, (c + 1) * 512)
                    egs = []
                    for g in range(EG):
                        k0, k1 = g * GK, min((g + 1) * GK, KC)
                        t = encs.tile([128, GK * 512], f32, tag="eg")
                        nc.sync.dma_start(
                            out=t[:, : (k1 - k0) * 512].rearrange(
                                "p (a n) -> p a n", a=k1 - k0
                            ),
                            in_=encp[k0 * 128 : k1 * 128, cs].rearrange(
                                "(a p) n -> p a n", p=128
                            ),
                        )
                        egs.append(t)
                    for r in range(R):
                        ps = eps.tile([128, 512], f32, tag=f"e{r}")
                        for kk in range(KC):
                            g, gi = kk // GK, kk % GK
                            nc.tensor.matmul(
                                ps,
                                xt_sb[:, kk * rows + r * 128 : kk * rows + (r + 1) * 128],
                                egs[g][:, gi * 512 : (gi + 1) * 512],
                                start=(kk == 0),
                                stop=(kk == KC - 1),
                            )
                        rt = evp.tile([128, 512], f32, tag="rt")
                        nc.scalar.activation(rt, ps, Relu)
                        rs = slice(r * 128, (r + 1) * 128)
                        nc.sync.dma_start(out=flat[rs, cs], in_=rt)
                        nc.sync.dma_start(out=scr[rs, cs], in_=rt)
                        nc.vector.max(out=cands[r][:, c * 8 : (c + 1) * 8], in_=rt)

            # ---------------- threshold merge + select + decode ----------------
            with (
                tc.tile_pool(name="post", bufs=1) as post,
                tc.tile_pool(name="mrg", bufs=2) as mrg,
                tc.tile_pool(name="tps", bufs=2, space="PSUM") as tpsp,
            ):
                ones_sb = post.tile([1, 128], bf16, tag="ones")
                nc.vector.memset(ones_sb, 1.0)
                pb_sb = post.tile([1, n_in], bf16, tag="pb")
                nc.sync.dma_start(out=pb_sb, in_=pb[:, :])

                for r in range(R):
                    mx = None
                    for ro in range(rounds):
                        mx = mrg.tile([128, 8], f32, tag="mx")
                        nc.vector.max(out=mx, in_=cands[r])
                        if ro < rounds - 1:
                            nc.vector.match_replace(
                                out=cands[r], in_to_replace=mx,
                                in_values=cands[r], imm_value=-3e38,
                            )
                    ti = (k - 1) - (rounds - 1) * 8
                    nc.vector.tensor_copy(thrs[r], mx[:, ti : ti + 1])

                for r in range(R):
                    rs = slice(r * 128, (r + 1) * 128)
                    with (
                        tc.tile_pool(name=f"latT{r}", bufs=1) as latTp,
                        tc.tile_pool(name=f"sel{r}", bufs=2) as selp,
                        tc.tile_pool(name=f"dec{r}", bufs=3) as decp,
                        tc.tile_pool(name=f"dev{r}", bufs=2) as devp,
                        tc.tile_pool(name=f"dps{r}", bufs=1, space="PSUM") as dps,
                    ):
                        latT = latTp.tile([128, L * 128], bf16, tag="latT")
                        flatT = latTp.tile([128, L * 128], bf16, tag="flatT")
                        # select: latents = flat * (flat >= t); transposes for decode
                        for s in range(SC):
                            ss = slice(s * SW, (s + 1) * SW)
                            fch = selp.tile([128, SW], f32, tag="fch")
                            nc.sync.dma_start(out=fch, in_=scr[rs, ss])
                            lch = selp.tile([128, SW], f32, tag="lch")
                            nc.vector.scalar_tensor_tensor(
                                out=lch, in0=fch, scalar=thrs[r], in1=fch,
                                op0=is_ge, op1=mult,
                            )
                            nc.sync.dma_start(out=lat[rs, ss], in_=lch)
                            for j in range(TPS):
                                l = s * TPS + j
                                js = slice(j * 128, (j + 1) * 128)
                                ls = slice(l * 128, (l + 1) * 128)
                                pt = tpsp.tile([128, 128], f32, tag="pt")
                                nc.tensor.transpose(pt, lch[:, js], ident)
                                nc.vector.tensor_copy(latT[:, ls], pt)
                                pt2 = tpsp.tile([128, 128], f32, tag="pt")
                                nc.tensor.transpose(pt2, fch[:, js], ident)
                                nc.vector.tensor_copy(flatT[:, ls], pt2)
                        # decode both reconstructions
                        for q0, qn in quads:
                            pls = [dps.tile([128, 512], f32, tag=f"pl{i}", name=f"pl{r}_{q0}_{i}") for i in range(qn)]
                            pfs = [dps.tile([128, 512], f32, tag=f"pf{i}", name=f"pf{r}_{q0}_{i}") for i in range(qn)]
                            for i in range(qn):
                                ncol = slice((q0 + i) * 512, (q0 + i + 1) * 512)
                                nc.tensor.matmul(pls[i], ones_sb, pb_sb[:, ncol],
                                                 start=True, stop=False)
                                nc.tensor.matmul(pfs[i], ones_sb, pb_sb[:, ncol],
                                                 start=True, stop=False)
                            for l0 in range(0, L, LG):
                                dch = decp.tile([128, LG * qn * 512], bf16, tag="dch")
                                nc.sync.dma_start(
                                    out=dch[:, :].rearrange("p (a n) -> p a n", a=LG),
                                    in_=dec[l0 * 128 : (l0 + LG) * 128,
                                            q0 * 512 : (q0 + qn) * 512].rearrange(
                                        "(a p) n -> p a n", p=128
                                    ),
                                )
                                for li in range(LG):
                                    l = l0 + li
                                    ls = slice(l * 128, (l + 1) * 128)
                                    stop = l == L - 1
                                    for i in range(qn):
                                        ds = slice((li * qn + i) * 512,
                                                   (li * qn + i + 1) * 512)
                                        nc.tensor.matmul(pls[i], latT[:, ls], dch[:, ds],
                                                         start=False, stop=stop)
                                    for i in range(qn):
                                        ds = slice((li * qn + i) * 512,
                                                   (li * qn + i + 1) * 512)
                                        nc.tensor.matmul(pfs[i], flatT[:, ls], dch[:, ds],
                                                         start=False, stop=stop)
                            for i in range(qn):
                                ncol = slice((q0 + i) * 512, (q0 + i + 1) * 512)
                                ev1 = devp.tile([128, 512], f32, tag="ev")
                                nc.scalar.activation(ev1, pls[i], Copy)
                                nc.sync.dma_start(out=recon[rs, ncol], in_=ev1)
                                ev2 = devp.tile([128, 512], f32, tag="ev")
                                nc.scalar.activation(ev2, pfs[i], Copy)
                                nc.sync.dma_start(out=recon_f[rs, ncol], in_=ev2)
    nc.compile()
    return nc


def _prep_inputs(x, pre_bias, encoder, latent_bias, decoder, rows, n_in, n_lat):
    """Host-side prep: fold biases, pad K, transpose x, cast decoder."""
    KC = n_in // 128 + 1
    n_in_pad = KC * 128
    n_cores = x.shape[0] // rows
    xb = (x - pre_bias[None, :]).astype(np.float32)
    encp = np.empty((n_in_pad, n_lat), dtype=np.float32)
    encp[:n_in] = encoder
    encp[n_in] = latent_bias
    encp[n_in + 1 :] = 0.0
    dec_bf = decoder.astype(ml_dtypes.bfloat16)
    pb2 = np.ascontiguousarray(pre_bias[None, :].astype(ml_dtypes.bfloat16))
    in_maps = []
    for c in range(n_cores):
        xTc = np.empty((n_in_pad, rows), dtype=np.float32)
        xTc[:n_in] = xb[c * rows : (c + 1) * rows].T
        xTc[n_in] = 1.0
        xTc[n_in + 1 :] = 0.0
        in_maps.append({"xT": xTc, "encp": encp, "dec": dec_bf, "pb": pb2})
    return in_maps


def kernel(x, pre_bias, encoder, latent_bias, decoder, k):
    from concourse.bass_utils import run_bass_kernel_spmd

    k = int(k)
    n_b, n_in = x.shape
    n_lat = encoder.shape[1]
    rows = n_b // N_CORES
    key = (rows, n_in, n_lat, k)
    if key not in _nc_cache:
        _nc_cache[key] = build_nc(rows, n_in, n_lat, k)
    nc = _nc_cache[key]
    in_maps = _prep_inputs(x, pre_bias, encoder, latent_bias, decoder,
                           rows, n_in, n_lat)
    res = run_bass_kernel_spmd(nc, in_maps, core_ids=list(range(N_CORES)))
    reconstructed = np.concatenate([r["recon"] for r in res.results], axis=0)
    latents = np.concatenate([r["lat"] for r in res.results], axis=0)
    reconstructed_full = np.concatenate([r["recon_f"] for r in res.results], axis=0)
    full_latents = np.concatenate([r["flat"] for r in res.results], axis=0)
    return reconstructed, latents, reconstructed_full, full_latents


# revision 4
# speedup vs baseline: 1.2343x; 1.2343x over previous
"""TopK sparse autoencoder kernel for Trainium2 (8 NeuronCores, data-parallel).

Problem: B=4096, N_IN=4096, N_LAT=32768, K=64
  pre   = (x - pre_bias) @ encoder + latent_bias          [B, N_LAT]
  latents = top-64-masked relu(pre); full_latents = relu(pre)
  reconstructed(_full) = latents(full) @ decoder + pre_bias

Sharding: data-parallel over batch. Each of 8 cores handles 512 rows with
replicated weights; no collectives.

Per-core device program:
  encode: fp32 GEMM (top-k selection is precision-critical; fp32r/bf16 would
          swap near-threshold selections vs the fp32 reference), bias folded
          in via an appended ones-row (host-side).
  top-k : per 512-wide output chunk, DVE max8 gives top-8 candidates
          (64 chunks x 8 = 512 candidates/row always contain the true
          top-64 up to ~1e-5/row probability); 8 rounds of max8 +
          match_replace on candidates give the 64th-largest threshold t;
          latents = flat * (flat >= t) where flat = relu(pre).
  decode: bf16 GEMMs with latents/full_latents transposed via PE.
"""
import numpy as np
import ml_dtypes

N_CORES = 8
B, N_IN, N_LAT, TOPK = 4096, 4096, 32768, 64

_nc_cache = {}
ENC_MODE = "bf16x3"  # "fp32" = exact-selection fallback


def build_nc(rows=B // N_CORES, n_in=N_IN, n_lat=N_LAT, k=TOPK):
    import concourse.mybir as mybir
    from concourse import bacc
    from concourse.tile import TileContext
    from concourse.masks import make_identity

    f32, bf16 = mybir.dt.float32, mybir.dt.bfloat16
    Relu = mybir.ActivationFunctionType.Relu
    Copy = mybir.ActivationFunctionType.Copy
    is_ge, mult = mybir.AluOpType.is_ge, mybir.AluOpType.mult

    KC = n_in // 128 + 1          # contraction chunks incl. bias row (padded)
    n_in_pad = KC * 128
    R = rows // 128               # rows-tiles
    C = n_lat // 512              # encode N-chunks
    L = n_lat // 128              # decode contraction chunks
    SW = 1024                     # select slice width
    SC = n_lat // SW
    TPS = SW // 128               # transposes per select slice
    GK = 9                        # encoder K-chunks per stream group
    EG = (KC + GK - 1) // GK      # stream groups per encode N-chunk
    LG = 2                        # decode l-chunks per dec DMA
    NINC = n_in // 512            # nin output chunks
    # decode psum quads: groups of nin chunks processed per pass (2 types each)
    quads = []
    q0 = 0
    while q0 < NINC:
        qn = min(3, NINC - q0)
        quads.append((q0, qn))
        q0 += qn
    rounds = (k + 7) // 8
    assert rows % 128 == 0 and n_lat % 512 == 0 and n_in % 512 == 0
    assert k <= C * 8, "candidate pool must cover k"

    nc = bacc.Bacc()
    if ENC_MODE == "bf16x3":
        xh_d = nc.dram_tensor("xh", [n_in_pad, rows], bf16, kind="ExternalInput")
        xl_d = nc.dram_tensor("xl", [n_in_pad, rows], bf16, kind="ExternalInput")
        eh_d = nc.dram_tensor("eh", [n_in_pad, n_lat], bf16, kind="ExternalInput")
        el_d = nc.dram_tensor("el", [n_in_pad, n_lat], bf16, kind="ExternalInput")
    else:
        xT = nc.dram_tensor("xT", [n_in_pad, rows], f32, kind="ExternalInput")
        encp = nc.dram_tensor("encp", [n_in_pad, n_lat], f32, kind="ExternalInput")
    dec = nc.dram_tensor("dec", [n_lat, n_in], bf16, kind="ExternalInput")
    pb = nc.dram_tensor("pb", [1, n_in], bf16, kind="ExternalInput")
    recon = nc.dram_tensor("recon", [rows, n_in], f32, kind="ExternalOutput")
    lat = nc.dram_tensor("lat", [rows, n_lat], f32, kind="ExternalOutput")
    recon_f = nc.dram_tensor("recon_f", [rows, n_in], f32, kind="ExternalOutput")
    flat = nc.dram_tensor("flat", [rows, n_lat], f32, kind="ExternalOutput")

    with TileContext(nc) as tc:
        with (
            tc.tile_pool(name="glob", bufs=1) as glob,
            tc.tile_pool(name="dram", bufs=1, space="DRAM") as dramp,
        ):
            ident = glob.tile([128, 128], f32, tag="ident")
            make_identity(nc, ident)
            cands = [glob.tile([128, C * 8], f32, tag=f"cand{r}", name=f"cand{r}") for r in range(R)]
            thrs = [glob.tile([128, 1], f32, tag=f"thr{r}", name=f"thr{r}") for r in range(R)]
            scr = dramp.tile([rows, n_lat], f32, tag="scr")  # relu(pre) scratch

            # ---------------- encode ----------------
            with (
                tc.tile_pool(name="xt", bufs=1) as xtp,
                tc.tile_pool(name="encs", bufs=4) as encs,
                tc.tile_pool(name="ev", bufs=4) as evp,
                tc.tile_pool(name="eps", bufs=2, space="PSUM") as eps,
            ):
                if ENC_MODE == "bf16x3":
                    xh_sb = xtp.tile([128, KC * rows], bf16, tag="xh")
                    xl_sb = xtp.tile([128, KC * rows], bf16, tag="xl")
                    nc.sync.dma_start(
                        out=xh_sb[:, :].rearrange("p (a m) -> p a m", a=KC),
                        in_=xh_d[:, :].rearrange("(a p) m -> p a m", p=128),
                    )
                    nc.sync.dma_start(
                        out=xl_sb[:, :].rearrange("p (a m) -> p a m", a=KC),
                        in_=xl_d[:, :].rearrange("(a p) m -> p a m", p=128),
                    )
                else:
                    xt_sb = xtp.tile([128, KC * rows], f32, tag="xt")
                    nc.sync.dma_start(
                        out=xt_sb[:, :].rearrange("p (a m) -> p a m", a=KC),
                        in_=xT[:, :].rearrange("(a p) m -> p a m", p=128),
                    )
                for c in range(C):
                    cs = slice(c * 512, (c + 1) * 512)
                    egs = []
                    for g in range(EG):
                        k0, k1 = g * GK, min((g + 1) * GK, KC)
                        if ENC_MODE == "bf16x3":
                            th = encs.tile([128, GK * 512], bf16, tag="egh", name=f"egh{c}_{g}")
                            tl = encs.tile([128, GK * 512], bf16, tag="egl", name=f"egl{c}_{g}")
                            for t, src in ((th, eh_d), (tl, el_d)):
                                nc.sync.dma_start(
                                    out=t[:, : (k1 - k0) * 512].rearrange(
                                        "p (a n) -> p a n", a=k1 - k0
                                    ),
                                    in_=src[k0 * 128 : k1 * 128, cs].rearrange(
                                        "(a p) n -> p a n", p=128
                                    ),
                                )
                            egs.append((th, tl))
                        else:
                            t = encs.tile([128, GK * 512], f32, tag="eg", name=f"eg{c}_{g}")
                            nc.sync.dma_start(
                                out=t[:, : (k1 - k0) * 512].rearrange(
                                    "p (a n) -> p a n", a=k1 - k0
                                ),
                                in_=encp[k0 * 128 : k1 * 128, cs].rearrange(
                                    "(a p) n -> p a n", p=128
                                ),
                            )
                            egs.append(t)
                    for r in range(R):
                        ps = eps.tile([128, 512], f32, tag=f"e{r}")
                        for kk in range(KC):
                            g, gi = kk // GK, kk % GK
                            xsl = slice(kk * rows + r * 128, kk * rows + (r + 1) * 128)
                            esl = slice(gi * 512, (gi + 1) * 512)
                            if ENC_MODE == "bf16x3":
                                th, tl = egs[g]
                                last = kk == KC - 1
                                nc.tensor.matmul(ps, xh_sb[:, xsl], th[:, esl],
                                                 start=(kk == 0), stop=False)
                                nc.tensor.matmul(ps, xh_sb[:, xsl], tl[:, esl],
                                                 start=False, stop=False)
                                nc.tensor.matmul(ps, xl_sb[:, xsl], th[:, esl],
                                                 start=False, stop=last)
                            else:
                                nc.tensor.matmul(
                                    ps, xt_sb[:, xsl], egs[g][:, esl],
                                    start=(kk == 0), stop=(kk == KC - 1),
                                )
                        rt = evp.tile([128, 512], f32, tag="rt")
                        nc.scalar.activation(rt, ps, Relu)
                        rs = slice(r * 128, (r + 1) * 128)
                        nc.sync.dma_start(out=flat[rs, cs], in_=rt)
                        nc.sync.dma_start(out=scr[rs, cs], in_=rt)
                        nc.vector.max(out=cands[r][:, c * 8 : (c + 1) * 8], in_=rt)

            # ---------------- threshold merge + select + decode ----------------
            with (
                tc.tile_pool(name="post", bufs=1) as post,
                tc.tile_pool(name="mrg", bufs=2) as mrg,
                tc.tile_pool(name="tps", bufs=2, space="PSUM") as tpsp,
            ):
                ones_sb = post.tile([1, 128], bf16, tag="ones")
                nc.vector.memset(ones_sb, 1.0)
                pb_sb = post.tile([1, n_in], bf16, tag="pb")
                nc.sync.dma_start(out=pb_sb, in_=pb[:, :])

                for r in range(R):
                    mx = None
                    for ro in range(rounds):
                        mx = mrg.tile([128, 8], f32, tag="mx")
                        nc.vector.max(out=mx, in_=cands[r])
                        if ro < rounds - 1:
                            nc.vector.match_replace(
                                out=cands[r], in_to_replace=mx,
                                in_values=cands[r], imm_value=-3e38,
                            )
                    ti = (k - 1) - (rounds - 1) * 8
                    nc.vector.tensor_copy(thrs[r], mx[:, ti : ti + 1])

                for r in range(R):
                    rs = slice(r * 128, (r + 1) * 128)
                    with (
                        tc.tile_pool(name=f"latT{r}", bufs=1) as latTp,
                        tc.tile_pool(name=f"sel{r}", bufs=2) as selp,
                        tc.tile_pool(name=f"dec{r}", bufs=3) as decp,
                        tc.tile_pool(name=f"dev{r}", bufs=2) as devp,
                        tc.tile_pool(name=f"dps{r}", bufs=1, space="PSUM") as dps,
                    ):
                        latT = latTp.tile([128, L * 128], bf16, tag="latT")
                        flatT = latTp.tile([128, L * 128], bf16, tag="flatT")
                        # select: latents = flat * (flat >= t); transposes for decode
                        for s in range(SC):
                            ss = slice(s * SW, (s + 1) * SW)
                            fch = selp.tile([128, SW], f32, tag="fch")
                            nc.sync.dma_start(out=fch, in_=scr[rs, ss])
                            lch = selp.tile([128, SW], f32, tag="lch")
                            nc.vector.scalar_tensor_tensor(
                                out=lch, in0=fch, scalar=thrs[r], in1=fch,
                                op0=is_ge, op1=mult,
                            )
                            nc.sync.dma_start(out=lat[rs, ss], in_=lch)
                            for j in range(TPS):
                                l = s * TPS + j
                                js = slice(j * 128, (j + 1) * 128)
                                ls = slice(l * 128, (l + 1) * 128)
                                pt = tpsp.tile([128, 128], f32, tag="pt")
                                nc.tensor.transpose(pt, lch[:, js], ident)
                                nc.vector.tensor_copy(latT[:, ls], pt)
                                pt2 = tpsp.tile([128, 128], f32, tag="pt")
                                nc.tensor.transpose(pt2, fch[:, js], ident)
                                nc.vector.tensor_copy(flatT[:, ls], pt2)
                        # decode both reconstructions
                        for q0, qn in quads:
                            pls = [dps.tile([128, 512], f32, tag=f"pl{i}", name=f"pl{r}_{q0}_{i}") for i in range(qn)]
                            pfs = [dps.tile([128, 512], f32, tag=f"pf{i}", name=f"pf{r}_{q0}_{i}") for i in range(qn)]
                            for i in range(qn):
                                ncol = slice((q0 + i) * 512, (q0 + i + 1) * 512)
                                nc.tensor.matmul(pls[i], ones_sb, pb_sb[:, ncol],
                                                 start=True, stop=False)
                                nc.tensor.matmul(pfs[i], ones_sb, pb_sb[:, ncol],
                                                 start=True, stop=False)
                            for l0 in range(0, L, LG):
                                dch = decp.tile([128, LG * qn * 512], bf16, tag="dch")
                                nc.sync.dma_start(
                                    out=dch[:, :].rearrange("p (a n) -> p a n", a=LG),
                                    in_=dec[l0 * 128 : (l0 + LG) * 128,
                                            q0 * 512 : (q0 + qn) * 512].rearrange(
                                        "(a p) n -> p a n", p=128
                                    ),
                                )
                                for li in range(LG):
                                    l = l0 + li
                                    ls = slice(l * 128, (l + 1) * 128)
                                    stop = l == L - 1
                                    for i in range(qn):
                                        ds = slice((li * qn + i) * 512,
                                                   (li * qn + i + 1) * 512)
                                        nc.tensor.matmul(pls[i], latT[:, ls], dch[:, ds],
                                                         start=False, stop=stop)
                                    for i in range(qn):
                                        ds = slice((li * qn + i) * 512,
                                                   (li * qn + i + 1) * 512)
                                        nc.tensor.matmul(pfs[i], flatT[:, ls], dch[:, ds],
                                                         start=False, stop=stop)
                            for i in range(qn):
                                ncol = slice((q0 + i) * 512, (q0 + i + 1) * 512)
                                ev1 = devp.tile([128, 512], f32, tag="ev")
                                nc.scalar.activation(ev1, pls[i], Copy)
                                nc.sync.dma_start(out=recon[rs, ncol], in_=ev1)
                                ev2 = devp.tile([128, 512], f32, tag="ev")
                                nc.scalar.activation(ev2, pfs[i], Copy)
                                nc.sync.dma_start(out=recon_f[rs, ncol], in_=ev2)
    nc.compile()
    return nc


def _prep_inputs(x, pre_bias, encoder, latent_bias, decoder, rows, n_in, n_lat):
    """Host-side prep: fold biases, pad K, transpose x, cast decoder."""
    KC = n_in // 128 + 1
    n_in_pad = KC * 128
    n_cores = x.shape[0] // rows
    xb = (x - pre_bias[None, :]).astype(np.float32)
    encp = np.empty((n_in_pad, n_lat), dtype=np.float32)
    encp[:n_in] = encoder
    encp[n_in] = latent_bias
    encp[n_in + 1 :] = 0.0
    dec_bf = decoder.astype(ml_dtypes.bfloat16)
    pb2 = np.ascontiguousarray(pre_bias[None, :].astype(ml_dtypes.bfloat16))
    if ENC_MODE == "bf16x3":
        eh = encp.astype(ml_dtypes.bfloat16)
        el = (encp - eh.astype(np.float32)).astype(ml_dtypes.bfloat16)
    in_maps = []
    for c in range(n_cores):
        xTc = np.empty((n_in_pad, rows), dtype=np.float32)
        xTc[:n_in] = xb[c * rows : (c + 1) * rows].T
        xTc[n_in] = 1.0
        xTc[n_in + 1 :] = 0.0
        if ENC_MODE == "bf16x3":
            xh = xTc.astype(ml_dtypes.bfloat16)
            xl = (xTc - xh.astype(np.float32)).astype(ml_dtypes.bfloat16)
            in_maps.append({"xh": xh, "xl": xl, "eh": eh, "el": el,
                            "dec": dec_bf, "pb": pb2})
        else:
            in_maps.append({"xT": xTc, "encp": encp, "dec": dec_bf, "pb": pb2})
    return in_maps


def kernel(x, pre_bias, encoder, latent_bias, decoder, k):
    from concourse.bass_utils import run_bass_kernel_spmd

    k = int(k)
    n_b, n_in = x.shape
    n_lat = encoder.shape[1]
    rows = n_b // N_CORES
    key = (rows, n_in, n_lat, k)
    if key not in _nc_cache:
        _nc_cache[key] = build_nc(rows, n_in, n_lat, k)
    nc = _nc_cache[key]
    in_maps = _prep_inputs(x, pre_bias, encoder, latent_bias, decoder,
                           rows, n_in, n_lat)
    res = run_bass_kernel_spmd(nc, in_maps, core_ids=list(range(N_CORES)))
    reconstructed = np.concatenate([r["recon"] for r in res.results], axis=0)
    latents = np.concatenate([r["lat"] for r in res.results], axis=0)
    reconstructed_full = np.concatenate([r["recon_f"] for r in res.results], axis=0)
    full_latents = np.concatenate([r["flat"] for r in res.results], axis=0)
    return reconstructed, latents, reconstructed_full, full_latents
